# revision 17
# baseline (speedup 1.0000x reference)
"""CTC compressor (weighted strategy) for Trainium2 — Bass/Tile kernel.

Problem: B=8, T=2048, D=1024, V=1024.
  probs = softmax(ctc_logits); pred = argmax(ctc_logits)
  segments = runs of equal non-blank pred within length; per-frame weight
  p[t] = probs[t, pred[t]] normalized within segment; output[s] = weighted
  sum of hidden over frames of segment s (zero-padded to T rows).

Key identity: out[s] = (sum_{t in seg s} p~[t] * h[t]) / (sum p~ + eps)
with p~ = p * frame_in_seg.  Segments are contiguous frame runs, so the
segment sums are differences of a global cumulative sum along T:
  S[s] = CE[a[s+1]] - CE[a[s]],  CE[t] = sum_{tau<t} p~ h,  a[s] = start of seg s.
This replaces the reference's dense (T x T') x (T x D) matmul (8.6 GFLOP/core)
with: softmax stats + hardware prefix-scans + indirect row gathers + a
banded-diff matmul, all memory-bound.

The cumsum runs in transposed layout ([d-part, t-free] prefix scan), and the
CE table is split into 4 d-pair tables so that scan -> transpose-back ->
CE write -> gather -> diff -> store pipelines per pair instead of
serializing on one full-width table.

Sharding: pure data parallel — one batch element per NeuronCore (8 cores).
"""

import numpy as np
from contextlib import ExitStack

import concourse.bass as bass
import concourse.bacc as bacc
import concourse.mybir as mybir
import concourse.tile as tile
from concourse.bass import IndirectOffsetOnAxis
from concourse.bass_utils import run_bass_kernel_spmd
from concourse.masks import make_identity

F32 = mybir.dt.float32
F32R = mybir.dt.float32r
I32 = mybir.dt.int32
U32 = mybir.dt.uint32
AF = mybir.ActivationFunctionType
OP = mybir.AluOpType

T, D, V = 2048, 1024, 1024
P = 128
NT = T // P            # 16 t-chunks
ND = D // P            # 8 d-chunks
NPAIR = 4              # d-chunk pairs; each CE table covers 256 dims
PW = 2 * P             # 256 dims per pair table
CW0 = PW + 16          # pair-0 table row: 256 dims + p~ col + 15 pad
PCOL = PW              # p~ cumsum column (pair-0 table only)
CEROWS = T + 1         # row 0 = zeros, row 1+t = inclusive cumsum through t
AROWS = 4224           # segment-start table; >= TRASH + T
TRASH = 2064.0         # masked scatter targets: rows TRASH + t (unique, unread)
EPS = 1e-10
NCORES = 8
GPS_SCANS = ()         # GpSimd cannot run scans (walrus rejects); all on DVE


def _build_body(ctx, tc, nc, lg, hs, lenb, out, nlen):
    sbc = ctx.enter_context(tc.tile_pool(name="sbc", bufs=1))
    sb = ctx.enter_context(tc.tile_pool(name="sb", bufs=2))
    sbio = ctx.enter_context(tc.tile_pool(name="sbio", bufs=3))
    sbot = ctx.enter_context(tc.tile_pool(name="sbot", bufs=6))
    sbz = ctx.enter_context(tc.tile_pool(name="sbz", bufs=1))
    sbg = ctx.enter_context(tc.tile_pool(name="sbg", bufs=10))
    pt = ctx.enter_context(tc.tile_pool(name="pt", bufs=2, space="PSUM"))
    ps = ctx.enter_context(tc.tile_pool(name="ps", bufs=4, space="PSUM"))
    dram = ctx.enter_context(tc.tile_pool(name="dram", bufs=1, space="DRAM"))

    # ---- constants ----
    ident = sbc.tile([P, P], F32, tag="ident")
    make_identity(nc, ident[:])

    # adiff[p, r] = +1 if p == r+1, -1 if p == r  -> out[r] = G[r+1] - G[r]
    adiff = sbc.tile([P, P], F32, tag="adiff")
    nc.gpsimd.memset(adiff[:], 0.0)
    nc.gpsimd.affine_select(out=adiff[:], in_=adiff[:], compare_op=OP.not_equal,
                            fill=-1.0, base=0, pattern=[[-1, P]], channel_multiplier=1)
    nc.gpsimd.affine_select(out=adiff[:], in_=adiff[:], compare_op=OP.not_equal,
                            fill=1.0, base=-1, pattern=[[-1, P]], channel_multiplier=1)

    tiota = sbc.tile([P, NT], I32, tag="tiota")
    nc.gpsimd.iota(tiota[:], pattern=[[P, NT]], base=0, channel_multiplier=1)
    tiotaf = sbc.tile([P, NT], F32, tag="tiotaf")
    nc.vector.tensor_copy(tiotaf[:], tiota[:])

    lent = sbc.tile([P, 1], F32, tag="lent")
    nc.sync.dma_start(lent[:], lenb[:])

    # ---- DRAM scratch ----
    cets = [dram.tile([CEROWS, CW0 if p == 0 else PW], F32, name=f"cet{p}",
                      tag=f"cet{p}") for p in range(NPAIR)]
    at = dram.tile([AROWS, 1], I32, name="at", tag="at")

    ainit = sbc.tile([P, AROWS // P], I32, tag="ainit")
    nc.gpsimd.memset(ainit[:], T)
    nc.sync.dma_start(at[:, :], ainit[:])

    zrow = sbc.tile([1, CW0], F32, tag="zrow")
    nc.gpsimd.memset(zrow[:], 0.0)
    for pr in range(NPAIR):
        nc.sync.dma_start(cets[pr][0:1, :], zrow[0:1, 0:(CW0 if pr == 0 else PW)])

    # persistent per-frame stat tiles ([128, 16]: t = partition + 128*chunk)
    mcol = sbc.tile([P, NT], F32, tag="mcol")    # negated max logit
    dns = sbc.tile([P, NT], F32, tag="dns")
    pcol = sbc.tile([P, NT], F32, tag="pcol")
    predf = sbc.tile([P, NT], F32, tag="predf")
    prevf = sbc.tile([P, NT], F32, tag="prevf")
    nb = sbc.tile([P, NT], F32, tag="nb")
    ptil = sbc.tile([P, NT], F32, tag="ptil")
    l00 = sbc.tile([1, 1], F32, tag="l00")

    # transposed z / cumsum tiles, one per d-chunk
    zTs = [sbz.tile([P, T], F32, name=f"zT{j}", tag=f"zT{j}") for j in range(ND)]

    # ---- fused per-t-chunk front pipeline ----
    for i in range(NT):
        lgt = sbio.tile([P, V], F32, tag="lgt")
        nc.sync.dma_start(lgt[:], lg[P * i:P * (i + 1), :])
        # per-frame max (GpSimd) + argmax (DVE) + exp-sum (ACT)
        mx8 = sb.tile([P, 8], F32, tag="mx8")
        nc.vector.max(mx8[:], lgt[:])
        ix8 = sb.tile([P, 8], U32, tag="ix8")
        nc.vector.max_index(ix8[:], mx8[:], lgt[:])
        nc.vector.tensor_scalar_mul(mcol[:, i:i + 1], mx8[:, 0:1], -1.0)
        nc.vector.tensor_copy(predf[:, i:i + 1], ix8[:, 0:1])
        nc.scalar.activation(lgt[:], lgt[:], AF.Exp, bias=mcol[:, i:i + 1],
                             scale=1.0, accum_out=dns[:, i:i + 1])
        nc.vector.reciprocal(pcol[:, i:i + 1], dns[:, i:i + 1])
        if i == 0:
            nc.vector.tensor_copy(l00[:], lgt[0:1, 0:1])

        # prev-pred shift for this chunk
        nc.scalar.dma_start(prevf[1:P, i:i + 1], predf[0:P - 1, i:i + 1])
        if i == 0:
            nc.gpsimd.memset(prevf[0:1, 0:1], -1.0)
        else:
            nc.scalar.dma_start(prevf[0:1, i:i + 1], predf[P - 1:P, i - 1:i])

        # masks -> nb (segment starts), ptil = p * frame_in_seg (GpSimd)
        vl = sb.tile([P, 1], F32, tag="vl")
        nc.vector.tensor_scalar(vl[:], tiotaf[:, i:i + 1], lent[:, 0:1], None,
                                op0=OP.is_lt)
        nq = sb.tile([P, 1], F32, tag="nq")
        nc.vector.tensor_tensor(nq[:], predf[:, i:i + 1], prevf[:, i:i + 1],
                                op=OP.not_equal)
        nbk = sb.tile([P, 1], F32, tag="nbk")
        nc.vector.tensor_scalar(nbk[:], predf[:, i:i + 1], 0.0, None,
                                op0=OP.not_equal)
        fi = sb.tile([P, 1], F32, tag="fi")
        nc.gpsimd.tensor_tensor(fi[:], vl[:], nbk[:], op=OP.mult)
        nc.gpsimd.tensor_tensor(ptil[:, i:i + 1], pcol[:, i:i + 1], fi[:],
                                op=OP.mult)
        nc.gpsimd.tensor_tensor(nq[:], nq[:], vl[:], op=OP.mult)
        nc.gpsimd.tensor_tensor(nb[:, i:i + 1], nq[:], nbk[:], op=OP.mult)

        # z_i = ptil_i * h_i, then transpose into per-d-chunk scan tiles
        ht = sbio.tile([P, D], F32, tag="ht")
        nc.sync.dma_start(ht[:], hs[P * i:P * (i + 1), :])
        if i == 0:
            hrow0 = sbc.tile([1, D], F32, tag="hrow0")
            nc.vector.tensor_copy(hrow0[:], ht[0:1, :])
        nc.vector.tensor_scalar_mul(ht[:], ht[:], ptil[:, i:i + 1])
        for g in range(2):
            tp = pt.tile([P, 512], F32, tag="tp")
            for q in range(4):
                j = 4 * g + q
                nc.tensor.transpose(tp[:, P * q:P * (q + 1)],
                                    ht[:, P * j:P * (j + 1)], ident[:])
            for q in range(4):
                j = 4 * g + q
                eng = nc.vector if (i + j) % 2 == 0 else nc.scalar
                if eng is nc.vector:
                    nc.vector.tensor_copy(zTs[j][:, P * i:P * (i + 1)],
                                          tp[:, P * q:P * (q + 1)])
                else:
                    nc.scalar.copy(zTs[j][:, P * i:P * (i + 1)],
                                   tp[:, P * q:P * (q + 1)])

    # ---- fold p~ and nb into rows; prefix-scan for seg ids / p cumsum ----
    pnb = sb.tile([P, 2 * NT], F32, tag="pnb")
    nc.vector.tensor_copy(pnb[:, 0:NT], ptil[:])
    nc.vector.tensor_copy(pnb[:, NT:2 * NT], nb[:])
    ps32 = pt.tile([2 * NT, P], F32, tag="tp")
    nc.tensor.transpose(ps32[:], pnb[:], ident[:])
    pnbT = sb.tile([2 * NT, P], F32, tag="pnbT")
    nc.vector.tensor_copy(pnbT[:], ps32[:])

    prow = sbc.tile([1, T], F32, tag="prow")
    nrow = sbc.tile([1, T], F32, tag="nrow")
    nc.sync.dma_start(prow[:], pnbT[0:NT, :])
    nc.sync.dma_start(nrow[:], pnbT[NT:2 * NT, :])

    pcrow = sbc.tile([1, T], F32, tag="pcrow")
    nc.vector.tensor_tensor_scan(pcrow[:], prow[:], prow[:], initial=0.0,
                                 op0=OP.add, op1=OP.bypass)
    scrow = sbc.tile([1, T], F32, tag="scrow")
    nc.vector.tensor_tensor_scan(scrow[:], nrow[:], nrow[:], initial=0.0,
                                 op0=OP.add, op1=OP.bypass)

    seg16 = sbc.tile([P, NT], F32, tag="seg16")
    for c in range(NT):
        nc.scalar.dma_start(seg16[:, c:c + 1], scrow[0:1, P * c:P * (c + 1)])

    # ---- scatter segment starts: a[seg_idx[t]] = t where nb[t] ----
    si = sb.tile([P, NT], F32, tag="si")
    nc.vector.tensor_scalar_add(si[:], seg16[:], -1.0)
    tr = sb.tile([P, NT], F32, tag="tr")
    nc.vector.tensor_scalar_add(tr[:], tiotaf[:], TRASH)
    idxf = sb.tile([P, NT], F32, tag="idxf")
    nc.vector.tensor_tensor(idxf[:], si[:], tr[:], op=OP.subtract)
    nc.vector.tensor_tensor(idxf[:], idxf[:], nb[:], op=OP.mult)
    nc.vector.tensor_tensor(idxf[:], idxf[:], tr[:], op=OP.add)
    idxi = sbc.tile([P, NT], I32, tag="idxi")
    nc.vector.tensor_copy(idxi[:], idxf[:])
    for c in range(NT):
        nc.gpsimd.indirect_dma_start(
            out=at[:, :],
            out_offset=IndirectOffsetOnAxis(ap=idxi[:, c:c + 1], axis=0),
            in_=tiota[:, c:c + 1],
            in_offset=None)

    # asb[p, c] = a[127*c + p]: each gather chunk holds 128 consecutive
    # segment starts, so all 127 adjacent diffs are intra-chunk (no seam fix)
    asb = sbc.tile([P, NT + 1], I32, tag="asb")
    for c in range(NT + 1):
        nc.scalar.dma_start(asb[:, c:c + 1], at[127 * c:127 * c + P, 0:1])

    # ---- global prefix scans along T (6 on DVE, 2 on GpSimd) ----
    for j in range(ND):
        eng = nc.gpsimd if j in GPS_SCANS else nc.vector
        eng.tensor_tensor_scan(zTs[j][:], zTs[j][:], zTs[j][:], initial=0.0,
                               op0=OP.add, op1=OP.bypass)

    # ---- per-pair: transpose back, append p-col (pair 0), write CE table ----
    for i in range(NT):
        for pr in range(NPAIR):
            w = CW0 if pr == 0 else PW
            ce = sbot.tile([P, CW0], F32, tag="ce")
            tp2 = pt.tile([P, PW], F32, tag="tp2")
            for h in range(2):
                j = 2 * pr + h
                nc.tensor.transpose(tp2[:, P * h:P * (h + 1)],
                                    zTs[j][:, P * i:P * (i + 1)], ident[:])
            if (i + pr) % 2 == 0:
                nc.vector.tensor_copy(ce[:, 0:PW], tp2[:])
            else:
                nc.scalar.copy(ce[:, 0:PW], tp2[:])
            if pr == 0:
                nc.gpsimd.memset(ce[:, PCOL:CW0], 0.0)
                nc.sync.dma_start(ce[:, PCOL:PCOL + 1],
                                  pcrow[0:1, P * i:P * (i + 1)])
            nc.sync.dma_start(cets[pr][1 + P * i:1 + P * (i + 1), :],
                              ce[:, 0:w])

    # ---- fallback + new_lengths scalars ----
    nsegv = scrow[0:1, T - 1:T]
    e00 = sb.tile([1, 1], F32, tag="e00")
    nc.scalar.activation(e00[:], l00[:], AF.Exp, bias=mcol[0:1, 0:1], scale=1.0)
    p0 = sb.tile([1, 1], F32, tag="p0")
    nc.vector.tensor_tensor(p0[:], e00[:], pcol[0:1, 0:1], op=OP.mult)
    pe0 = sb.tile([1, 1], F32, tag="pe0")
    nc.vector.tensor_scalar_add(pe0[:], p0[:], EPS)
    per0 = sb.tile([1, 1], F32, tag="per0")
    nc.vector.reciprocal(per0[:], pe0[:])
    fc = sb.tile([1, 1], F32, tag="fc")
    nc.vector.tensor_tensor(fc[:], p0[:], per0[:], op=OP.mult)
    e1 = sb.tile([1, 1], F32, tag="e1")
    nc.vector.tensor_scalar(e1[:], nsegv, 0.0, None, op0=OP.is_equal)
    e2 = sb.tile([1, 1], F32, tag="e2")
    nc.vector.tensor_scalar(e2[:], lent[0:1, 0:1], 1.0, None, op0=OP.is_ge)
    flag = sb.tile([1, 1], F32, tag="flag")
    nc.vector.tensor_tensor(flag[:], e1[:], e2[:], op=OP.mult)
    fcoef = sbc.tile([1, 1], F32, tag="fcoef")
    nc.vector.tensor_tensor(fcoef[:], fc[:], flag[:], op=OP.mult)
    fbrow = sbc.tile([1, D], F32, tag="fbrow")
    nc.vector.tensor_scalar_mul(fbrow[:], hrow0[:], fcoef[0:1, 0:1])

    nlf = sb.tile([1, 1], F32, tag="nlf")
    nc.vector.tensor_scalar_max(nlf[:], nsegv, 1.0)
    nli = sb.tile([1, 1], I32, tag="nli")
    nc.vector.tensor_copy(nli[:], nlf[:])
    nc.sync.dma_start(nlen[:, :], nli[:])

    # ---- gather G[s] = CE[a[s]] per pair, band-diff, normalize, store ----
    gts = {}
    for pr in range(NPAIR):
        w = CW0 if pr == 0 else PW
        for k in range(NT + 1):
            gt = sbg.tile([P, w], F32, tag=f"gt{pr}")
            nc.gpsimd.indirect_dma_start(
                out=gt[:], out_offset=None, in_=cets[pr][:, :],
                in_offset=IndirectOffsetOnAxis(ap=asb[:, k:k + 1], axis=0))
            gts[(pr, k)] = gt

    recs = {}
    for k in range(NT + 1):
        n = min(127, T - 127 * k)          # output rows this chunk
        for pr in range(NPAIR):
            w = CW0 if pr == 0 else PW
            sp = ps.tile([P, CW0], F32, tag="sp")
            nc.tensor.matmul(sp[:, 0:w], lhsT=adiff[:], rhs=gts[(pr, k)][:, 0:w],
                             start=True, stop=True)
            if pr == 0:
                radd = sb.tile([P, 1], F32, tag="radd")
                nc.vector.tensor_scalar_add(radd[:], sp[:, PCOL:PCOL + 1], EPS)
                rec = sbot.tile([P, 1], F32, tag="rec")
                nc.vector.reciprocal(rec[:], radd[:])
                recs[k] = rec
            ot = sbot.tile([P, PW], F32, tag="ot")
            if (k + pr) % 2 == 0:
                nc.scalar.mul(ot[:], sp[:, 0:PW], recs[k][:])
            else:
                nc.vector.tensor_scalar_mul(ot[:], sp[:, 0:PW], recs[k][:])
            if k == 0:
                nc.vector.tensor_tensor(ot[0:1, :], ot[0:1, :],
                                        fbrow[0:1, PW * pr:PW * (pr + 1)],
                                        op=OP.add)
            nc.sync.dma_start(
                out[127 * k:127 * k + n, PW * pr:PW * (pr + 1)], ot[0:n, :])


def build_nc():
    nc = bacc.Bacc("TRN2", target_bir_lowering=False, debug=False)
    lg = nc.dram_tensor("lg", [T, V], F32, kind="ExternalInput")
    hs = nc.dram_tensor("hs", [T, D], F32, kind="ExternalInput")
    lenb = nc.dram_tensor("lenb", [P, 1], F32, kind="ExternalInput")
    out = nc.dram_tensor("out", [T, D], F32, kind="ExternalOutput")
    nlen = nc.dram_tensor("nlen", [1, 1], I32, kind="ExternalOutput")
    with tile.TileContext(nc) as tc:
        with ExitStack() as ctx:
            _build_body(ctx, tc, nc, lg.ap(), hs.ap(), lenb.ap(), out.ap(),
                        nlen.ap())
    nc.compile()
    return nc


_NC = None


def _get_nc():
    global _NC
    if _NC is None:
        _NC = build_nc()
    return _NC


def make_in_maps(hidden_states, ctc_logits, lengths):
    in_maps = []
    for b in range(NCORES):
        in_maps.append({
            "lg": np.ascontiguousarray(ctc_logits[b], dtype=np.float32),
            "hs": np.ascontiguousarray(hidden_states[b], dtype=np.float32),
            "lenb": np.full((P, 1), float(lengths[b]), dtype=np.float32),
        })
    return in_maps


def kernel(hidden_states, ctc_logits, lengths, **run_kwargs):
    hidden_states = np.asarray(hidden_states)
    ctc_logits = np.asarray(ctc_logits)
    lengths = np.asarray(lengths)
    nc = _get_nc()
    in_maps = make_in_maps(hidden_states, ctc_logits, lengths)
    res = run_bass_kernel_spmd(nc, in_maps, core_ids=list(range(NCORES)),
                               **run_kwargs)
    compressed = np.stack([res.results[b]["out"] for b in range(NCORES)])
    new_lengths = np.array(
        [res.results[b]["nlen"].reshape(()) for b in range(NCORES)],
        dtype=np.int32)
    return compressed, new_lengths


# revision 18
# speedup vs baseline: 1.0547x; 1.0547x over previous
"""CTC compressor (weighted strategy) for Trainium2 — Bass/Tile kernel.

Problem: B=8, T=2048, D=1024, V=1024.
  probs = softmax(ctc_logits); pred = argmax(ctc_logits)
  segments = runs of equal non-blank pred within length; per-frame weight
  p[t] = probs[t, pred[t]] normalized within segment; output[s] = weighted
  sum of hidden over frames of segment s (zero-padded to T rows).

Key identity: out[s] = (sum_{t in seg s} p~[t] * h[t]) / (sum p~ + eps)
with p~ = p * frame_in_seg.  Segments are contiguous frame runs, so the
segment sums are differences of a global cumulative sum along T:
  S[s] = CE[a[s+1]] - CE[a[s]],  CE[t] = sum_{tau<t} p~ h,  a[s] = start of seg s.
This replaces the reference's dense (T x T') x (T x D) matmul (8.6 GFLOP/core)
with: softmax stats + hardware prefix-scans + indirect row gathers + a
banded-diff matmul, all memory-bound.

The cumsum runs in transposed layout ([d-part, t-free] prefix scan), and the
CE table is split into 4 d-pair tables so that scan -> transpose-back ->
CE write -> gather -> diff -> store pipelines per pair instead of
serializing on one full-width table.

Sharding: pure data parallel — one batch element per NeuronCore (8 cores).
"""

import numpy as np
from contextlib import ExitStack

import concourse.bass as bass
import concourse.bacc as bacc
import concourse.mybir as mybir
import concourse.tile as tile
from concourse.bass import IndirectOffsetOnAxis
from concourse.bass_utils import run_bass_kernel_spmd
from concourse.masks import make_identity

F32 = mybir.dt.float32
F32R = mybir.dt.float32r
I32 = mybir.dt.int32
U32 = mybir.dt.uint32
AF = mybir.ActivationFunctionType
OP = mybir.AluOpType

T, D, V = 2048, 1024, 1024
P = 128
NT = T // P            # 16 t-chunks
ND = D // P            # 8 d-chunks
NPAIR = 2              # d-chunk groups; each CE table covers 512 dims
PW = 4 * P             # 512 dims per group table
CW0 = PW + 16          # group-0 table row: 512 dims + p~ col + 15 pad
PCOL = PW              # p~ cumsum column (pair-0 table only)
CEROWS = T + 1         # row 0 = zeros, row 1+t = inclusive cumsum through t
AROWS = 4224           # segment-start table; >= TRASH + T
TRASH = 2064.0         # masked scatter targets: rows TRASH + t (unique, unread)
EPS = 1e-10
NCORES = 8
GPS_SCANS = ()         # GpSimd cannot run scans (walrus rejects); all on DVE


def _build_body(ctx, tc, nc, lg, hs, lenb, out, nlen):
    sbc = ctx.enter_context(tc.tile_pool(name="sbc", bufs=1))
    sb = ctx.enter_context(tc.tile_pool(name="sb", bufs=2))
    sbio = ctx.enter_context(tc.tile_pool(name="sbio", bufs=3))
    sbot = ctx.enter_context(tc.tile_pool(name="sbot", bufs=3))
    sbz = ctx.enter_context(tc.tile_pool(name="sbz", bufs=1))
    sbg = ctx.enter_context(tc.tile_pool(name="sbg", bufs=4))
    pt = ctx.enter_context(tc.tile_pool(name="pt", bufs=2, space="PSUM"))
    ps = ctx.enter_context(tc.tile_pool(name="ps", bufs=3, space="PSUM"))
    dram = ctx.enter_context(tc.tile_pool(name="dram", bufs=1, space="DRAM"))

    # ---- constants ----
    ident = sbc.tile([P, P], F32, tag="ident")
    make_identity(nc, ident[:])

    # adiff[p, r] = +1 if p == r+1, -1 if p == r  -> out[r] = G[r+1] - G[r]
    adiff = sbc.tile([P, P], F32, tag="adiff")
    nc.gpsimd.memset(adiff[:], 0.0)
    nc.gpsimd.affine_select(out=adiff[:], in_=adiff[:], compare_op=OP.not_equal,
                            fill=-1.0, base=0, pattern=[[-1, P]], channel_multiplier=1)
    nc.gpsimd.affine_select(out=adiff[:], in_=adiff[:], compare_op=OP.not_equal,
                            fill=1.0, base=-1, pattern=[[-1, P]], channel_multiplier=1)

    tiota = sbc.tile([P, NT], I32, tag="tiota")
    nc.gpsimd.iota(tiota[:], pattern=[[P, NT]], base=0, channel_multiplier=1)
    tiotaf = sbc.tile([P, NT], F32, tag="tiotaf")
    nc.vector.tensor_copy(tiotaf[:], tiota[:])

    lent = sbc.tile([P, 1], F32, tag="lent")
    nc.sync.dma_start(lent[:], lenb[:])

    # ---- DRAM scratch ----
    cets = [dram.tile([CEROWS, CW0 if p == 0 else PW], F32, name=f"cet{p}",
                      tag=f"cet{p}") for p in range(NPAIR)]
    at = dram.tile([AROWS, 1], I32, name="at", tag="at")

    ainit = sbc.tile([P, AROWS // P], I32, tag="ainit")
    nc.gpsimd.memset(ainit[:], T)
    nc.sync.dma_start(at[:, :], ainit[:])

    zrow = sbc.tile([1, CW0], F32, tag="zrow")
    nc.gpsimd.memset(zrow[:], 0.0)
    for pr in range(NPAIR):
        nc.sync.dma_start(cets[pr][0:1, :], zrow[0:1, 0:(CW0 if pr == 0 else PW)])

    # persistent per-frame stat tiles ([128, 16]: t = partition + 128*chunk)
    mcol = sbc.tile([P, NT], F32, tag="mcol")    # negated max logit
    dns = sbc.tile([P, NT], F32, tag="dns")
    pcol = sbc.tile([P, NT], F32, tag="pcol")
    predf = sbc.tile([P, NT], F32, tag="predf")
    prevf = sbc.tile([P, NT], F32, tag="prevf")
    nb = sbc.tile([P, NT], F32, tag="nb")
    ptil = sbc.tile([P, NT], F32, tag="ptil")
    l00 = sbc.tile([1, 1], F32, tag="l00")

    # transposed z / cumsum tiles, one per 4-d-chunk group
    zTp = [sbz.tile([P, 4 * T], F32, name=f"zTp{g}", tag=f"zTp{g}")
           for g in range(NPAIR)]

    # ---- fused per-t-chunk front pipeline ----
    for i in range(NT):
        lgt = sbio.tile([P, V], F32, tag="lgt")
        nc.sync.dma_start(lgt[:], lg[P * i:P * (i + 1), :])
        # per-frame max (GpSimd) + argmax (DVE) + exp-sum (ACT)
        mx8 = sb.tile([P, 8], F32, tag="mx8")
        nc.vector.max(mx8[:], lgt[:])
        ix8 = sb.tile([P, 8], U32, tag="ix8")
        nc.vector.max_index(ix8[:], mx8[:], lgt[:])
        nc.vector.tensor_scalar_mul(mcol[:, i:i + 1], mx8[:, 0:1], -1.0)
        nc.vector.tensor_copy(predf[:, i:i + 1], ix8[:, 0:1])
        nc.scalar.activation(lgt[:], lgt[:], AF.Exp, bias=mcol[:, i:i + 1],
                             scale=1.0, accum_out=dns[:, i:i + 1])
        nc.vector.reciprocal(pcol[:, i:i + 1], dns[:, i:i + 1])
        if i == 0:
            nc.vector.tensor_copy(l00[:], lgt[0:1, 0:1])

        # masks (valid, nonblank) -> ptil = p * frame_in_seg
        vl = sb.tile([P, 1], F32, tag="vl")
        nc.vector.tensor_scalar(vl[:], tiotaf[:, i:i + 1], lent[:, 0:1], None,
                                op0=OP.is_lt)
        nbk = sb.tile([P, 1], F32, tag="nbk")
        nc.vector.tensor_scalar(nbk[:], predf[:, i:i + 1], 0.0, None,
                                op0=OP.not_equal)
        fi = sb.tile([P, 1], F32, tag="fi")
        nc.gpsimd.tensor_tensor(fi[:], vl[:], nbk[:], op=OP.mult)
        nc.gpsimd.tensor_tensor(ptil[:, i:i + 1], pcol[:, i:i + 1], fi[:],
                                op=OP.mult)

        # z_i = ptil_i * h_i, then transpose into the group scan tiles
        ht = sbio.tile([P, D], F32, tag="ht")
        nc.sync.dma_start(ht[:], hs[P * i:P * (i + 1), :])
        if i == 0:
            hrow0 = sbc.tile([1, D], F32, tag="hrow0")
            nc.vector.tensor_copy(hrow0[:], ht[0:1, :])
        nc.vector.tensor_scalar_mul(ht[:], ht[:], ptil[:, i:i + 1])
        for g in range(NPAIR):
            tp = pt.tile([P, 512], F32, tag="tp")
            for q in range(4):
                j = 4 * g + q
                nc.tensor.transpose(tp[:, P * q:P * (q + 1)],
                                    ht[:, P * j:P * (j + 1)], ident[:])
            dst = zTp[g][:].rearrange("p (j t) -> p j t", j=4)[:, :,
                                                              P * i:P * (i + 1)]
            srcv = tp[:].rearrange("p (q t) -> p q t", q=4)
            if (i + g) % 2 == 0:
                nc.vector.tensor_copy(dst, srcv)
            else:
                nc.scalar.copy(dst, srcv)

    # ---- run boundaries (whole-tile prev shift) -> nb ----
    nc.scalar.dma_start(prevf[1:P, :], predf[0:P - 1, :])
    nc.scalar.dma_start(prevf[0:1, 1:NT], predf[P - 1:P, 0:NT - 1])
    nc.gpsimd.memset(prevf[0:1, 0:1], -1.0)
    nbq = sb.tile([P, NT], F32, tag="nbq")
    nc.vector.tensor_tensor(nbq[:], predf[:], prevf[:], op=OP.not_equal)
    vall = sb.tile([P, NT], F32, tag="vall")
    nc.vector.tensor_scalar(vall[:], tiotaf[:], lent[:, 0:1], None, op0=OP.is_lt)
    nbl = sb.tile([P, NT], F32, tag="nbl")
    nc.vector.tensor_scalar(nbl[:], predf[:], 0.0, None, op0=OP.not_equal)
    nc.vector.tensor_tensor(nbq[:], nbq[:], vall[:], op=OP.mult)
    nc.vector.tensor_tensor(nb[:], nbq[:], nbl[:], op=OP.mult)

    # ---- fold p~ and nb into rows; prefix-scan for seg ids / p cumsum ----
    pnb = sb.tile([P, 2 * NT], F32, tag="pnb")
    nc.vector.tensor_copy(pnb[:, 0:NT], ptil[:])
    nc.vector.tensor_copy(pnb[:, NT:2 * NT], nb[:])
    ps32 = pt.tile([2 * NT, P], F32, tag="tp")
    nc.tensor.transpose(ps32[:], pnb[:], ident[:])
    pnbT = sb.tile([2 * NT, P], F32, tag="pnbT")
    nc.vector.tensor_copy(pnbT[:], ps32[:])

    prow = sbc.tile([1, T], F32, tag="prow")
    nrow = sbc.tile([1, T], F32, tag="nrow")
    nc.sync.dma_start(prow[:], pnbT[0:NT, :])
    nc.sync.dma_start(nrow[:], pnbT[NT:2 * NT, :])

    pcrow = sbc.tile([1, T], F32, tag="pcrow")
    nc.vector.tensor_tensor_scan(pcrow[:], prow[:], prow[:], initial=0.0,
                                 op0=OP.add, op1=OP.bypass)
    scrow = sbc.tile([1, T], F32, tag="scrow")
    nc.vector.tensor_tensor_scan(scrow[:], nrow[:], nrow[:], initial=0.0,
                                 op0=OP.add, op1=OP.bypass)

    seg16 = sbc.tile([P, NT], F32, tag="seg16")
    for c in range(NT):
        nc.scalar.dma_start(seg16[:, c:c + 1], scrow[0:1, P * c:P * (c + 1)])

    # ---- scatter segment starts: a[seg_idx[t]] = t where nb[t] ----
    si = sb.tile([P, NT], F32, tag="si")
    nc.vector.tensor_scalar_add(si[:], seg16[:], -1.0)
    tr = sb.tile([P, NT], F32, tag="tr")
    nc.vector.tensor_scalar_add(tr[:], tiotaf[:], TRASH)
    idxf = sb.tile([P, NT], F32, tag="idxf")
    nc.vector.tensor_tensor(idxf[:], si[:], tr[:], op=OP.subtract)
    nc.vector.tensor_tensor(idxf[:], idxf[:], nb[:], op=OP.mult)
    nc.vector.tensor_tensor(idxf[:], idxf[:], tr[:], op=OP.add)
    idxi = sbc.tile([P, NT], I32, tag="idxi")
    nc.vector.tensor_copy(idxi[:], idxf[:])
    for c in range(NT):
        nc.gpsimd.indirect_dma_start(
            out=at[:, :],
            out_offset=IndirectOffsetOnAxis(ap=idxi[:, c:c + 1], axis=0),
            in_=tiota[:, c:c + 1],
            in_offset=None)

    # asb[p, c] = a[127*c + p]: each gather chunk holds 128 consecutive
    # segment starts, so all 127 adjacent diffs are intra-chunk (no seam fix)
    asb = sbc.tile([P, NT + 1], I32, tag="asb")
    for c in range(NT + 1):
        nc.scalar.dma_start(asb[:, c:c + 1], at[127 * c:127 * c + P, 0:1])

    # ---- global prefix scans along T (DVE only) ----
    for j in range(ND):
        g, h = divmod(j, 4)
        sl = zTp[g][:, T * h:T * (h + 1)]
        nc.vector.tensor_tensor_scan(sl, sl, sl, initial=0.0,
                                     op0=OP.add, op1=OP.bypass)

    # ---- per-group: transpose back, append p-col (group 0), write CE ----
    for i in range(NT):
        for pr in range(NPAIR):
            w = CW0 if pr == 0 else PW
            ce = sbot.tile([P, CW0], F32, tag="ce")
            tp2 = pt.tile([P, 512], F32, tag="tp")
            for h in range(4):
                nc.tensor.transpose(tp2[:, P * h:P * (h + 1)],
                                    zTp[pr][:, T * h + P * i:T * h + P * (i + 1)],
                                    ident[:])
            if (i + pr) % 2 == 0:
                nc.vector.tensor_copy(ce[:, 0:PW], tp2[:])
            else:
                nc.scalar.copy(ce[:, 0:PW], tp2[:])
            if pr == 0:
                nc.gpsimd.memset(ce[:, PCOL:CW0], 0.0)
                nc.scalar.dma_start(ce[:, PCOL:PCOL + 1],
                                    pcrow[0:1, P * i:P * (i + 1)])
            eng = nc.sync if pr == 0 else nc.scalar
            eng.dma_start(cets[pr][1 + P * i:1 + P * (i + 1), :], ce[:, 0:w])

    # ---- fallback + new_lengths scalars ----
    nsegv = scrow[0:1, T - 1:T]
    e00 = sb.tile([1, 1], F32, tag="e00")
    nc.scalar.activation(e00[:], l00[:], AF.Exp, bias=mcol[0:1, 0:1], scale=1.0)
    p0 = sb.tile([1, 1], F32, tag="p0")
    nc.vector.tensor_tensor(p0[:], e00[:], pcol[0:1, 0:1], op=OP.mult)
    pe0 = sb.tile([1, 1], F32, tag="pe0")
    nc.vector.tensor_scalar_add(pe0[:], p0[:], EPS)
    per0 = sb.tile([1, 1], F32, tag="per0")
    nc.vector.reciprocal(per0[:], pe0[:])
    fc = sb.tile([1, 1], F32, tag="fc")
    nc.vector.tensor_tensor(fc[:], p0[:], per0[:], op=OP.mult)
    e1 = sb.tile([1, 1], F32, tag="e1")
    nc.vector.tensor_scalar(e1[:], nsegv, 0.0, None, op0=OP.is_equal)
    e2 = sb.tile([1, 1], F32, tag="e2")
    nc.vector.tensor_scalar(e2[:], lent[0:1, 0:1], 1.0, None, op0=OP.is_ge)
    flag = sb.tile([1, 1], F32, tag="flag")
    nc.vector.tensor_tensor(flag[:], e1[:], e2[:], op=OP.mult)
    fcoef = sbc.tile([1, 1], F32, tag="fcoef")
    nc.vector.tensor_tensor(fcoef[:], fc[:], flag[:], op=OP.mult)
    fbrow = sbc.tile([1, D], F32, tag="fbrow")
    nc.vector.tensor_scalar_mul(fbrow[:], hrow0[:], fcoef[0:1, 0:1])

    nlf = sb.tile([1, 1], F32, tag="nlf")
    nc.vector.tensor_scalar_max(nlf[:], nsegv, 1.0)
    nli = sb.tile([1, 1], I32, tag="nli")
    nc.vector.tensor_copy(nli[:], nlf[:])
    nc.sync.dma_start(nlen[:, :], nli[:])

    # ---- gather G[s] = CE[a[s]] per pair, band-diff, normalize, store ----
    gts = {}
    for pr in range(NPAIR):
        w = CW0 if pr == 0 else PW
        for k in range(NT + 1):
            gt = sbg.tile([P, w], F32, tag=f"gt{pr}")
            nc.gpsimd.indirect_dma_start(
                out=gt[:], out_offset=None, in_=cets[pr][:, :],
                in_offset=IndirectOffsetOnAxis(ap=asb[:, k:k + 1], axis=0))
            gts[(pr, k)] = gt

    recs = {}
    for k in range(NT + 1):
        n = min(127, T - 127 * k)          # output rows this chunk
        obig = sbot.tile([P, D], F32, tag="obig")
        for pr in range(NPAIR):
            w = CW0 if pr == 0 else PW
            sp = ps.tile([P, CW0], F32, tag="sp")
            nc.tensor.matmul(sp[:, 0:512], lhsT=adiff[:],
                             rhs=gts[(pr, k)][:, 0:512], start=True, stop=True)
            if pr == 0:
                nc.tensor.matmul(sp[:, 512:CW0], lhsT=adiff[:],
                                 rhs=gts[(pr, k)][:, 512:CW0],
                                 start=True, stop=True)
                radd = sb.tile([P, 1], F32, tag="radd")
                nc.vector.tensor_scalar_add(radd[:], sp[:, PCOL:PCOL + 1], EPS)
                rec = sbot.tile([P, 1], F32, tag="rec")
                nc.vector.reciprocal(rec[:], radd[:])
                recs[k] = rec
            osl = obig[:, PW * pr:PW * (pr + 1)]
            if (k + pr) % 2 == 0:
                nc.scalar.mul(osl, sp[:, 0:PW], recs[k][:])
            else:
                nc.vector.tensor_scalar_mul(osl, sp[:, 0:PW], recs[k][:])
            if k == 0:
                nc.vector.tensor_tensor(obig[0:1, PW * pr:PW * (pr + 1)],
                                        obig[0:1, PW * pr:PW * (pr + 1)],
                                        fbrow[0:1, PW * pr:PW * (pr + 1)],
                                        op=OP.add)
        nc.sync.dma_start(out[127 * k:127 * k + n, :], obig[0:n, :])


def build_nc():
    nc = bacc.Bacc("TRN2", target_bir_lowering=False, debug=False)
    lg = nc.dram_tensor("lg", [T, V], F32, kind="ExternalInput")
    hs = nc.dram_tensor("hs", [T, D], F32, kind="ExternalInput")
    lenb = nc.dram_tensor("lenb", [P, 1], F32, kind="ExternalInput")
    out = nc.dram_tensor("out", [T, D], F32, kind="ExternalOutput")
    nlen = nc.dram_tensor("nlen", [1, 1], I32, kind="ExternalOutput")
    with tile.TileContext(nc) as tc:
        with ExitStack() as ctx:
            _build_body(ctx, tc, nc, lg.ap(), hs.ap(), lenb.ap(), out.ap(),
                        nlen.ap())
    nc.compile()
    return nc


_NC = None


def _get_nc():
    global _NC
    if _NC is None:
        _NC = build_nc()
    return _NC


def make_in_maps(hidden_states, ctc_logits, lengths):
    in_maps = []
    for b in range(NCORES):
        in_maps.append({
            "lg": np.ascontiguousarray(ctc_logits[b], dtype=np.float32),
            "hs": np.ascontiguousarray(hidden_states[b], dtype=np.float32),
            "lenb": np.full((P, 1), float(lengths[b]), dtype=np.float32),
        })
    return in_maps


def kernel(hidden_states, ctc_logits, lengths, **run_kwargs):
    hidden_states = np.asarray(hidden_states)
    ctc_logits = np.asarray(ctc_logits)
    lengths = np.asarray(lengths)
    nc = _get_nc()
    in_maps = make_in_maps(hidden_states, ctc_logits, lengths)
    res = run_bass_kernel_spmd(nc, in_maps, core_ids=list(range(NCORES)),
                               **run_kwargs)
    compressed = np.stack([res.results[b]["out"] for b in range(NCORES)])
    new_lengths = np.array(
        [res.results[b]["nlen"].reshape(()) for b in range(NCORES)],
        dtype=np.int32)
    return compressed, new_lengths


# revision 19
# speedup vs baseline: 1.7425x; 1.6521x over previous
"""CTC compressor (weighted strategy) for Trainium2 — Bass/Tile kernel.

Problem: B=8, T=2048, D=1024, V=1024.
  probs = softmax(ctc_logits); pred = argmax(ctc_logits)
  segments = runs of equal non-blank pred within length; per-frame weight
  p[t] = probs[t, pred[t]] normalized within segment; output[s] = weighted
  sum of hidden over frames of segment s (zero-padded to T rows).

Key identity: out[s] = (sum_{t in seg s} p~[t] * h[t]) / (sum p~ + eps)
with p~ = p * frame_in_seg.  Segments are contiguous frame runs, so the
segment sums are differences of a global cumulative sum along T:
  S[s] = CE[a[s+1]] - CE[a[s]],  CE[t] = sum_{tau<t} p~ h,  a[s] = start of seg s.
This replaces the reference's dense (T x T') x (T x D) matmul (8.6 GFLOP/core)
with: softmax stats + hardware prefix-scans + indirect row gathers + a
banded-diff matmul, all memory-bound.

The cumsum runs in transposed layout ([d-part, t-free] prefix scan), and the
CE table is split into 4 d-pair tables so that scan -> transpose-back ->
CE write -> gather -> diff -> store pipelines per pair instead of
serializing on one full-width table.

Sharding: pure data parallel — one batch element per NeuronCore (8 cores).
"""

import numpy as np
from contextlib import ExitStack

import concourse.bass as bass
import concourse.bacc as bacc
import concourse.mybir as mybir
import concourse.tile as tile
from concourse.bass import IndirectOffsetOnAxis
from concourse.bass_utils import run_bass_kernel_spmd
from concourse.masks import make_identity

F32 = mybir.dt.float32
F32R = mybir.dt.float32r
I32 = mybir.dt.int32
U32 = mybir.dt.uint32
AF = mybir.ActivationFunctionType
OP = mybir.AluOpType

T, D, V = 2048, 1024, 1024
P = 128
NT = T // P            # 16 t-chunks
ND = D // P            # 8 d-chunks
NPAIR = 2              # d-chunk groups; each CE table covers 512 dims
PW = 4 * P             # 512 dims per group table
CW0 = PW + 16          # group-0 table row: 512 dims + p~ col + 15 pad
PCOL = PW              # p~ cumsum column (pair-0 table only)
CEROWS = T + 1         # row 0 = zeros, row 1+t = inclusive cumsum through t
AROWS = 4224           # segment-start table; >= TRASH + T
TRASH = 2064.0         # masked scatter targets: rows TRASH + t (unique, unread)
EPS = 1e-10
NCORES = 8
GPS_SCANS = ()         # GpSimd cannot run scans (walrus rejects); all on DVE


def _build_body(ctx, tc, nc, lg, hs, lenb, out, nlen):
    sbc = ctx.enter_context(tc.tile_pool(name="sbc", bufs=1))
    sb = ctx.enter_context(tc.tile_pool(name="sb", bufs=2))
    sbio = ctx.enter_context(tc.tile_pool(name="sbio", bufs=3))
    sbot = ctx.enter_context(tc.tile_pool(name="sbot", bufs=3))
    sbz = ctx.enter_context(tc.tile_pool(name="sbz", bufs=1))
    sbg = ctx.enter_context(tc.tile_pool(name="sbg", bufs=4))
    pt = ctx.enter_context(tc.tile_pool(name="pt", bufs=2, space="PSUM"))
    ps = ctx.enter_context(tc.tile_pool(name="ps", bufs=3, space="PSUM"))
    dram = ctx.enter_context(tc.tile_pool(name="dram", bufs=1, space="DRAM"))

    # ---- constants ----
    ident = sbc.tile([P, P], F32, tag="ident")
    make_identity(nc, ident[:])

    # adiff[p, r] = +1 if p == r+1, -1 if p == r  -> out[r] = G[r+1] - G[r]
    adiff = sbc.tile([P, P], F32, tag="adiff")
    nc.gpsimd.memset(adiff[:], 0.0)
    nc.gpsimd.affine_select(out=adiff[:], in_=adiff[:], compare_op=OP.not_equal,
                            fill=-1.0, base=0, pattern=[[-1, P]], channel_multiplier=1)
    nc.gpsimd.affine_select(out=adiff[:], in_=adiff[:], compare_op=OP.not_equal,
                            fill=1.0, base=-1, pattern=[[-1, P]], channel_multiplier=1)

    tiota = sbc.tile([P, NT], I32, tag="tiota")
    nc.gpsimd.iota(tiota[:], pattern=[[P, NT]], base=0, channel_multiplier=1)
    tiotaf = sbc.tile([P, NT], F32, tag="tiotaf")
    nc.vector.tensor_copy(tiotaf[:], tiota[:])

    lent = sbc.tile([P, 1], F32, tag="lent")
    nc.sync.dma_start(lent[:], lenb[:])

    # ---- DRAM scratch ----
    cets = [dram.tile([CEROWS, CW0 if p == 0 else PW], F32, name=f"cet{p}",
                      tag=f"cet{p}") for p in range(NPAIR)]
    at = dram.tile([AROWS, 1], I32, name="at", tag="at")

    ainit = sbc.tile([P, AROWS // P], I32, tag="ainit")
    nc.gpsimd.memset(ainit[:], T)
    nc.sync.dma_start(at[:, :], ainit[:])

    zrow = sbc.tile([1, CW0], F32, tag="zrow")
    nc.gpsimd.memset(zrow[:], 0.0)
    for pr in range(NPAIR):
        nc.sync.dma_start(cets[pr][0:1, :], zrow[0:1, 0:(CW0 if pr == 0 else PW)])

    # persistent per-frame stat tiles ([128, 16]: t = partition + 128*chunk)
    mcol = sbc.tile([P, NT], F32, tag="mcol")    # negated max logit
    dns = sbc.tile([P, NT], F32, tag="dns")
    pcol = sbc.tile([P, NT], F32, tag="pcol")
    predf = sbc.tile([P, NT], F32, tag="predf")
    prevf = sbc.tile([P, NT], F32, tag="prevf")
    nb = sbc.tile([P, NT], F32, tag="nb")
    ptil = sbc.tile([P, NT], F32, tag="ptil")
    l00 = sbc.tile([1, 1], F32, tag="l00")

    # transposed z / cumsum tiles, one per 4-d-chunk group
    zTp = [sbz.tile([P, 4 * T], F32, name=f"zTp{g}", tag=f"zTp{g}")
           for g in range(NPAIR)]

    # ---- fused per-t-chunk front pipeline ----
    for i in range(NT):
        lgt = sbio.tile([P, V], F32, tag="lgt")
        nc.sync.dma_start(lgt[:], lg[P * i:P * (i + 1), :])
        # per-frame max (GpSimd) + argmax (DVE) + exp-sum (ACT)
        mx8 = sb.tile([P, 8], F32, tag="mx8")
        nc.vector.max(mx8[:], lgt[:])
        ix8 = sb.tile([P, 8], U32, tag="ix8")
        nc.vector.max_index(ix8[:], mx8[:], lgt[:])
        nc.vector.tensor_scalar_mul(mcol[:, i:i + 1], mx8[:, 0:1], -1.0)
        nc.vector.tensor_copy(predf[:, i:i + 1], ix8[:, 0:1])
        nc.scalar.activation(lgt[:], lgt[:], AF.Exp, bias=mcol[:, i:i + 1],
                             scale=1.0, accum_out=dns[:, i:i + 1])
        nc.vector.reciprocal(pcol[:, i:i + 1], dns[:, i:i + 1])
        if i == 0:
            nc.vector.tensor_copy(l00[:], lgt[0:1, 0:1])

        # masks (valid, nonblank) -> ptil = p * frame_in_seg
        vl = sb.tile([P, 1], F32, tag="vl")
        nc.vector.tensor_scalar(vl[:], tiotaf[:, i:i + 1], lent[:, 0:1], None,
                                op0=OP.is_lt)
        nbk = sb.tile([P, 1], F32, tag="nbk")
        nc.vector.tensor_scalar(nbk[:], predf[:, i:i + 1], 0.0, None,
                                op0=OP.not_equal)
        fi = sb.tile([P, 1], F32, tag="fi")
        nc.gpsimd.tensor_tensor(fi[:], vl[:], nbk[:], op=OP.mult)
        nc.gpsimd.tensor_tensor(ptil[:, i:i + 1], pcol[:, i:i + 1], fi[:],
                                op=OP.mult)

        # z_i = ptil_i * h_i, then transpose into the group scan tiles
        ht = sbio.tile([P, D], F32, tag="ht")
        nc.sync.dma_start(ht[:], hs[P * i:P * (i + 1), :])
        if i == 0:
            hrow0 = sbc.tile([1, D], F32, tag="hrow0")
            nc.vector.tensor_copy(hrow0[:], ht[0:1, :])
        nc.vector.tensor_scalar_mul(ht[:], ht[:], ptil[:, i:i + 1])
        for g in range(NPAIR):
            tp = pt.tile([P, 512], F32, tag="tp")
            for q in range(4):
                j = 4 * g + q
                nc.tensor.transpose(tp[:, P * q:P * (q + 1)],
                                    ht[:, P * j:P * (j + 1)], ident[:])
            dst = zTp[g][:].rearrange("p (j t) -> p j t", j=4)[:, :,
                                                              P * i:P * (i + 1)]
            srcv = tp[:].rearrange("p (q t) -> p q t", q=4)
            if (i + g) % 2 == 0:
                nc.vector.tensor_copy(dst, srcv)
            else:
                nc.scalar.copy(dst, srcv)

    # ---- run boundaries (whole-tile prev shift) -> nb ----
    nc.scalar.dma_start(prevf[1:P, :], predf[0:P - 1, :])
    nc.scalar.dma_start(prevf[0:1, 1:NT], predf[P - 1:P, 0:NT - 1])
    nc.gpsimd.memset(prevf[0:1, 0:1], -1.0)
    nbq = sb.tile([P, NT], F32, tag="nbq")
    nc.vector.tensor_tensor(nbq[:], predf[:], prevf[:], op=OP.not_equal)
    vall = sb.tile([P, NT], F32, tag="vall")
    nc.vector.tensor_scalar(vall[:], tiotaf[:], lent[:, 0:1], None, op0=OP.is_lt)
    nbl = sb.tile([P, NT], F32, tag="nbl")
    nc.vector.tensor_scalar(nbl[:], predf[:], 0.0, None, op0=OP.not_equal)
    nc.vector.tensor_tensor(nbq[:], nbq[:], vall[:], op=OP.mult)
    nc.vector.tensor_tensor(nb[:], nbq[:], nbl[:], op=OP.mult)

    # ---- fold p~ and nb into rows; prefix-scan for seg ids / p cumsum ----
    pnb = sb.tile([P, 2 * NT], F32, tag="pnb")
    nc.vector.tensor_copy(pnb[:, 0:NT], ptil[:])
    nc.vector.tensor_copy(pnb[:, NT:2 * NT], nb[:])
    ps32 = pt.tile([2 * NT, P], F32, tag="tp")
    nc.tensor.transpose(ps32[:], pnb[:], ident[:])
    pnbT = sb.tile([2 * NT, P], F32, tag="pnbT")
    nc.vector.tensor_copy(pnbT[:], ps32[:])

    prow = sbc.tile([1, T], F32, tag="prow")
    nrow = sbc.tile([1, T], F32, tag="nrow")
    nc.sync.dma_start(prow[:], pnbT[0:NT, :])
    nc.sync.dma_start(nrow[:], pnbT[NT:2 * NT, :])

    pcrow = sbc.tile([1, T], F32, tag="pcrow")
    nc.vector.tensor_tensor_scan(pcrow[:], prow[:], prow[:], initial=0.0,
                                 op0=OP.add, op1=OP.bypass)
    scrow = sbc.tile([1, T], F32, tag="scrow")
    nc.vector.tensor_tensor_scan(scrow[:], nrow[:], nrow[:], initial=0.0,
                                 op0=OP.add, op1=OP.bypass)

    seg16 = sbc.tile([P, NT], F32, tag="seg16")
    for c in range(NT):
        nc.scalar.dma_start(seg16[:, c:c + 1], scrow[0:1, P * c:P * (c + 1)])

    # ---- scatter segment starts: a[seg_idx[t]] = t where nb[t] ----
    si = sb.tile([P, NT], F32, tag="si")
    nc.vector.tensor_scalar_add(si[:], seg16[:], -1.0)
    tr = sb.tile([P, NT], F32, tag="tr")
    nc.vector.tensor_scalar_add(tr[:], tiotaf[:], TRASH)
    idxf = sb.tile([P, NT], F32, tag="idxf")
    nc.vector.tensor_tensor(idxf[:], si[:], tr[:], op=OP.subtract)
    nc.vector.tensor_tensor(idxf[:], idxf[:], nb[:], op=OP.mult)
    nc.vector.tensor_tensor(idxf[:], idxf[:], tr[:], op=OP.add)
    idxi = sbc.tile([P, NT], I32, tag="idxi")
    nc.vector.tensor_copy(idxi[:], idxf[:])
    for c in range(NT):
        nc.gpsimd.indirect_dma_start(
            out=at[:, :],
            out_offset=IndirectOffsetOnAxis(ap=idxi[:, c:c + 1], axis=0),
            in_=tiota[:, c:c + 1],
            in_offset=None)

    # asb[p, c] = a[127*c + p]: each gather chunk holds 128 consecutive
    # segment starts, so all 127 adjacent diffs are intra-chunk (no seam fix)
    asb = sbc.tile([P, NT + 1], I32, tag="asb")
    for c in range(NT + 1):
        nc.scalar.dma_start(asb[:, c:c + 1], at[127 * c:127 * c + P, 0:1])

    # ---- global prefix scans along T (DVE only) ----
    for j in range(ND):
        g, h = divmod(j, 4)
        sl = zTp[g][:, T * h:T * (h + 1)]
        nc.vector.tensor_tensor_scan(sl, sl, sl, initial=0.0,
                                     op0=OP.add, op1=OP.bypass)

    # ---- per-group: transpose back, append p-col (group 0), write CE ----
    for i in range(NT):
        for pr in range(NPAIR):
            w = CW0 if pr == 0 else PW
            ce = sbot.tile([P, CW0], F32, tag="ce")
            tp2 = pt.tile([P, 512], F32, tag="tp")
            for h in range(4):
                nc.tensor.transpose(tp2[:, P * h:P * (h + 1)],
                                    zTp[pr][:, T * h + P * i:T * h + P * (i + 1)],
                                    ident[:])
            if (i + pr) % 2 == 0:
                nc.vector.tensor_copy(ce[:, 0:PW], tp2[:])
            else:
                nc.scalar.copy(ce[:, 0:PW], tp2[:])
            if pr == 0:
                nc.gpsimd.memset(ce[:, PCOL:CW0], 0.0)
                nc.scalar.dma_start(ce[:, PCOL:PCOL + 1],
                                    pcrow[0:1, P * i:P * (i + 1)])
            eng = nc.sync if pr == 0 else nc.scalar
            eng.dma_start(cets[pr][1 + P * i:1 + P * (i + 1), :], ce[:, 0:w])

    # ---- fallback + new_lengths scalars ----
    nsegv = scrow[0:1, T - 1:T]
    e00 = sb.tile([1, 1], F32, tag="e00")
    nc.scalar.activation(e00[:], l00[:], AF.Exp, bias=mcol[0:1, 0:1], scale=1.0)
    p0 = sb.tile([1, 1], F32, tag="p0")
    nc.vector.tensor_tensor(p0[:], e00[:], pcol[0:1, 0:1], op=OP.mult)
    pe0 = sb.tile([1, 1], F32, tag="pe0")
    nc.vector.tensor_scalar_add(pe0[:], p0[:], EPS)
    per0 = sb.tile([1, 1], F32, tag="per0")
    nc.vector.reciprocal(per0[:], pe0[:])
    fc = sb.tile([1, 1], F32, tag="fc")
    nc.vector.tensor_tensor(fc[:], p0[:], per0[:], op=OP.mult)
    e1 = sb.tile([1, 1], F32, tag="e1")
    nc.vector.tensor_scalar(e1[:], nsegv, 0.0, None, op0=OP.is_equal)
    e2 = sb.tile([1, 1], F32, tag="e2")
    nc.vector.tensor_scalar(e2[:], lent[0:1, 0:1], 1.0, None, op0=OP.is_ge)
    flag = sb.tile([1, 1], F32, tag="flag")
    nc.vector.tensor_tensor(flag[:], e1[:], e2[:], op=OP.mult)
    fcoef = sbc.tile([1, 1], F32, tag="fcoef")
    nc.vector.tensor_tensor(fcoef[:], fc[:], flag[:], op=OP.mult)
    fbrow = sbc.tile([1, D], F32, tag="fbrow")
    nc.vector.tensor_scalar_mul(fbrow[:], hrow0[:], fcoef[0:1, 0:1])

    nlf = sb.tile([1, 1], F32, tag="nlf")
    nc.vector.tensor_scalar_max(nlf[:], nsegv, 1.0)
    nli = sb.tile([1, 1], I32, tag="nli")
    nc.vector.tensor_copy(nli[:], nlf[:])
    nc.sync.dma_start(nlen[:, :], nli[:])

    # ---- gather G[s] = CE[a[s]] per pair, band-diff, normalize, store ----
    gts = {}
    for pr in range(NPAIR):
        w = CW0 if pr == 0 else PW
        for k in range(NT + 1):
            gt = sbg.tile([P, w], F32, tag=f"gt{pr}")
            nc.gpsimd.indirect_dma_start(
                out=gt[:], out_offset=None, in_=cets[pr][:, :],
                in_offset=IndirectOffsetOnAxis(ap=asb[:, k:k + 1], axis=0))
            gts[(pr, k)] = gt

    recs = {}
    for k in range(NT + 1):
        n = min(127, T - 127 * k)          # output rows this chunk
        obig = sbot.tile([P, D], F32, tag="obig")
        for pr in range(NPAIR):
            w = CW0 if pr == 0 else PW
            sp = ps.tile([P, CW0], F32, tag="sp")
            nc.tensor.matmul(sp[:, 0:512], lhsT=adiff[:],
                             rhs=gts[(pr, k)][:, 0:512], start=True, stop=True)
            if pr == 0:
                nc.tensor.matmul(sp[:, 512:CW0], lhsT=adiff[:],
                                 rhs=gts[(pr, k)][:, 512:CW0],
                                 start=True, stop=True)
                radd = sb.tile([P, 1], F32, tag="radd")
                nc.vector.tensor_scalar_add(radd[:], sp[:, PCOL:PCOL + 1], EPS)
                rec = sbot.tile([P, 1], F32, tag="rec")
                nc.vector.reciprocal(rec[:], radd[:])
                recs[k] = rec
            osl = obig[:, PW * pr:PW * (pr + 1)]
            if (k + pr) % 2 == 0:
                nc.scalar.mul(osl, sp[:, 0:PW], recs[k][:])
            else:
                nc.vector.tensor_scalar_mul(osl, sp[:, 0:PW], recs[k][:])
            if k == 0:
                nc.vector.tensor_tensor(obig[0:1, PW * pr:PW * (pr + 1)],
                                        obig[0:1, PW * pr:PW * (pr + 1)],
                                        fbrow[0:1, PW * pr:PW * (pr + 1)],
                                        op=OP.add)
        # write 128 rows (full-partition source spreads across all 16 SDMA
        # engines; 127-row sources collapse onto one). Row 127 of the chunk
        # is garbage but is overwritten by chunk k+1's row 0 — same HWDGE
        # queue, FIFO order guarantees the final value.
        n128 = min(P, T - 127 * k)
        nc.sync.dma_start(out[127 * k:127 * k + n128, :], obig[0:n128, :])


def build_nc():
    nc = bacc.Bacc("TRN2", target_bir_lowering=False, debug=False)
    lg = nc.dram_tensor("lg", [T, V], F32, kind="ExternalInput")
    hs = nc.dram_tensor("hs", [T, D], F32, kind="ExternalInput")
    lenb = nc.dram_tensor("lenb", [P, 1], F32, kind="ExternalInput")
    out = nc.dram_tensor("out", [T, D], F32, kind="ExternalOutput")
    nlen = nc.dram_tensor("nlen", [1, 1], I32, kind="ExternalOutput")
    with tile.TileContext(nc) as tc:
        with ExitStack() as ctx:
            _build_body(ctx, tc, nc, lg.ap(), hs.ap(), lenb.ap(), out.ap(),
                        nlen.ap())
    nc.compile()
    return nc


_NC = None


def _get_nc():
    global _NC
    if _NC is None:
        _NC = build_nc()
    return _NC


def make_in_maps(hidden_states, ctc_logits, lengths):
    in_maps = []
    for b in range(NCORES):
        in_maps.append({
            "lg": np.ascontiguousarray(ctc_logits[b], dtype=np.float32),
            "hs": np.ascontiguousarray(hidden_states[b], dtype=np.float32),
            "lenb": np.full((P, 1), float(lengths[b]), dtype=np.float32),
        })
    return in_maps


def kernel(hidden_states, ctc_logits, lengths, **run_kwargs):
    hidden_states = np.asarray(hidden_states)
    ctc_logits = np.asarray(ctc_logits)
    lengths = np.asarray(lengths)
    nc = _get_nc()
    in_maps = make_in_maps(hidden_states, ctc_logits, lengths)
    res = run_bass_kernel_spmd(nc, in_maps, core_ids=list(range(NCORES)),
                               **run_kwargs)
    compressed = np.stack([res.results[b]["out"] for b in range(NCORES)])
    new_lengths = np.array(
        [res.results[b]["nlen"].reshape(()) for b in range(NCORES)],
        dtype=np.int32)
    return compressed, new_lengths


# revision 21
# speedup vs baseline: 2.1046x; 1.2078x over previous
"""CTC compressor (weighted strategy) for Trainium2 — Bass/Tile kernel.

Problem: B=8, T=2048, D=1024, V=1024.
  probs = softmax(ctc_logits); pred = argmax(ctc_logits)
  segments = runs of equal non-blank pred within length; per-frame weight
  p[t] = probs[t, pred[t]] normalized within segment; output[s] = weighted
  sum of hidden over frames of segment s (zero-padded to T rows).

Key identity: out[s] = (sum_{t in seg s} p~[t] * h[t]) / (sum p~ + eps)
with p~ = p * frame_in_seg.  Segments are contiguous frame runs, so the
segment sums are differences of a global cumulative sum along T:
  S[s] = CE[a[s+1]] - CE[a[s]],  CE[t] = sum_{tau<t} p~ h,  a[s] = start of seg s.
This replaces the reference's dense (T x T') x (T x D) matmul (8.6 GFLOP/core)
with: softmax stats + hardware prefix-scans + indirect row gathers + a
banded-diff matmul, all memory-bound.

The cumsum runs in transposed layout ([d-part, t-free] prefix scan), and the
CE table is split into 4 d-pair tables so that scan -> transpose-back ->
CE write -> gather -> diff -> store pipelines per pair instead of
serializing on one full-width table.

Sharding: pure data parallel — one batch element per NeuronCore (8 cores).
"""

import numpy as np
from contextlib import ExitStack

import concourse.bass as bass
import concourse.bacc as bacc
import concourse.mybir as mybir
import concourse.tile as tile
from concourse.bass import IndirectOffsetOnAxis
from concourse.bass_utils import run_bass_kernel_spmd
from concourse.masks import make_identity

F32 = mybir.dt.float32
F32R = mybir.dt.float32r
I32 = mybir.dt.int32
U32 = mybir.dt.uint32
AF = mybir.ActivationFunctionType
OP = mybir.AluOpType

T, D, V = 2048, 1024, 1024
P = 128
NT = T // P            # 16 t-chunks
ND = D // P            # 8 d-chunks
NPAIR = 2              # d-chunk groups; each CE table covers 512 dims
PW = 4 * P             # 512 dims per group table
CW0 = PW + 16          # group-0 table row: 512 dims + p~ col + 15 pad
PCOL = PW              # p~ cumsum column (pair-0 table only)
CEROWS = T + 1         # row 0 = zeros, row 1+t = inclusive cumsum through t
AROWS = 4224           # segment-start table; >= TRASH + T
TRASH = 2064.0         # masked scatter targets: rows TRASH + t (unique, unread)
EPS = 1e-10
NCORES = 8
GPS_SCANS = ()         # GpSimd cannot run scans (walrus rejects); all on DVE


def _build_body(ctx, tc, nc, lg, hs, lenb, out, nlen):
    sbc = ctx.enter_context(tc.tile_pool(name="sbc", bufs=1))
    sb = ctx.enter_context(tc.tile_pool(name="sb", bufs=2))
    sbio = ctx.enter_context(tc.tile_pool(name="sbio", bufs=3))
    sbot = ctx.enter_context(tc.tile_pool(name="sbot", bufs=3))
    sbz = ctx.enter_context(tc.tile_pool(name="sbz", bufs=1))
    sbg = ctx.enter_context(tc.tile_pool(name="sbg", bufs=4))
    pt = ctx.enter_context(tc.tile_pool(name="pt", bufs=2, space="PSUM"))
    ps = ctx.enter_context(tc.tile_pool(name="ps", bufs=3, space="PSUM"))
    dram = ctx.enter_context(tc.tile_pool(name="dram", bufs=1, space="DRAM"))

    # ---- constants ----
    ident = sbc.tile([P, P], F32, tag="ident")
    make_identity(nc, ident[:])

    # adiff[p, r] = +1 if p == r+1, -1 if p == r  -> out[r] = G[r+1] - G[r]
    adiff = sbc.tile([P, P], F32, tag="adiff")
    nc.gpsimd.memset(adiff[:], 0.0)
    nc.gpsimd.affine_select(out=adiff[:], in_=adiff[:], compare_op=OP.not_equal,
                            fill=-1.0, base=0, pattern=[[-1, P]], channel_multiplier=1)
    nc.gpsimd.affine_select(out=adiff[:], in_=adiff[:], compare_op=OP.not_equal,
                            fill=1.0, base=-1, pattern=[[-1, P]], channel_multiplier=1)

    tiota = sbc.tile([P, NT], I32, tag="tiota")
    nc.gpsimd.iota(tiota[:], pattern=[[P, NT]], base=0, channel_multiplier=1)
    tiotaf = sbc.tile([P, NT], F32, tag="tiotaf")
    nc.vector.tensor_copy(tiotaf[:], tiota[:])

    lent = sbc.tile([P, 1], F32, tag="lent")
    nc.sync.dma_start(lent[:], lenb[:])

    # ---- DRAM scratch ----
    cets = [dram.tile([CEROWS, CW0 if p == 0 else PW], F32, name=f"cet{p}",
                      tag=f"cet{p}") for p in range(NPAIR)]
    at = dram.tile([AROWS, 1], I32, name="at", tag="at")

    ainit = sbc.tile([P, AROWS // P], I32, tag="ainit")
    nc.gpsimd.memset(ainit[:], T)
    nc.sync.dma_start(at[:, :], ainit[:])

    zrow = sbc.tile([1, CW0], F32, tag="zrow")
    nc.gpsimd.memset(zrow[:], 0.0)
    for pr in range(NPAIR):
        nc.sync.dma_start(cets[pr][0:1, :], zrow[0:1, 0:(CW0 if pr == 0 else PW)])

    # persistent per-frame stat tiles ([128, 16]: t = partition + 128*chunk)
    mcol = sbc.tile([P, NT], F32, tag="mcol")    # negated max logit
    dns = sbc.tile([P, NT], F32, tag="dns")
    pcol = sbc.tile([P, NT], F32, tag="pcol")
    predf = sbc.tile([P, NT], F32, tag="predf")
    prevf = sbc.tile([P, NT], F32, tag="prevf")
    nb = sbc.tile([P, NT], F32, tag="nb")
    ptil = sbc.tile([P, NT], F32, tag="ptil")
    l00 = sbc.tile([1, 1], F32, tag="l00")

    # transposed z / cumsum tiles, one per 4-d-chunk group
    zTp = [sbz.tile([P, 4 * T], F32, name=f"zTp{g}", tag=f"zTp{g}")
           for g in range(NPAIR)]

    # ---- fused per-t-chunk front pipeline ----
    for i in range(NT):
        lgt = sbio.tile([P, V], F32, tag="lgt")
        nc.sync.dma_start(lgt[:], lg[P * i:P * (i + 1), :])
        # per-frame max (GpSimd) + argmax (DVE) + exp-sum (ACT)
        mx8 = sb.tile([P, 8], F32, tag="mx8")
        nc.vector.max(mx8[:], lgt[:])
        ix8 = sb.tile([P, 8], U32, tag="ix8")
        nc.vector.max_index(ix8[:], mx8[:], lgt[:])
        nc.vector.tensor_scalar_mul(mcol[:, i:i + 1], mx8[:, 0:1], -1.0)
        nc.vector.tensor_copy(predf[:, i:i + 1], ix8[:, 0:1])
        nc.scalar.activation(lgt[:], lgt[:], AF.Exp, bias=mcol[:, i:i + 1],
                             scale=1.0, accum_out=dns[:, i:i + 1])
        nc.vector.reciprocal(pcol[:, i:i + 1], dns[:, i:i + 1])
        if i == 0:
            nc.vector.tensor_copy(l00[:], lgt[0:1, 0:1])

        # masks (valid, nonblank) -> ptil = p * frame_in_seg
        vl = sb.tile([P, 1], F32, tag="vl")
        nc.vector.tensor_scalar(vl[:], tiotaf[:, i:i + 1], lent[:, 0:1], None,
                                op0=OP.is_lt)
        nbk = sb.tile([P, 1], F32, tag="nbk")
        nc.vector.tensor_scalar(nbk[:], predf[:, i:i + 1], 0.0, None,
                                op0=OP.not_equal)
        fi = sb.tile([P, 1], F32, tag="fi")
        nc.gpsimd.tensor_tensor(fi[:], vl[:], nbk[:], op=OP.mult)
        nc.gpsimd.tensor_tensor(ptil[:, i:i + 1], pcol[:, i:i + 1], fi[:],
                                op=OP.mult)

        # z_i = ptil_i * h_i, then transpose into the group scan tiles
        ht = sbio.tile([P, D], F32, tag="ht")
        nc.sync.dma_start(ht[:], hs[P * i:P * (i + 1), :])
        if i == 0:
            hrow0 = sbc.tile([1, D], F32, tag="hrow0")
            nc.vector.tensor_copy(hrow0[:], ht[0:1, :])
        nc.vector.tensor_scalar_mul(ht[:], ht[:], ptil[:, i:i + 1])
        for g in range(NPAIR):
            tp = pt.tile([P, 512], F32, tag="tp")
            for q in range(4):
                j = 4 * g + q
                nc.tensor.transpose(tp[:, P * q:P * (q + 1)],
                                    ht[:, P * j:P * (j + 1)], ident[:])
            dst = zTp[g][:].rearrange("p (j t) -> p j t", j=4)[:, :,
                                                              P * i:P * (i + 1)]
            srcv = tp[:].rearrange("p (q t) -> p q t", q=4)
            if (i + g) % 2 == 0:
                nc.vector.tensor_copy(dst, srcv)
            else:
                nc.scalar.copy(dst, srcv)

    # ---- run boundaries (whole-tile prev shift) -> nb ----
    nc.scalar.dma_start(prevf[1:P, :], predf[0:P - 1, :])
    nc.scalar.dma_start(prevf[0:1, 1:NT], predf[P - 1:P, 0:NT - 1])
    nc.gpsimd.memset(prevf[0:1, 0:1], -1.0)
    nbq = sb.tile([P, NT], F32, tag="nbq")
    nc.vector.tensor_tensor(nbq[:], predf[:], prevf[:], op=OP.not_equal)
    vall = sb.tile([P, NT], F32, tag="vall")
    nc.vector.tensor_scalar(vall[:], tiotaf[:], lent[:, 0:1], None, op0=OP.is_lt)
    nbl = sb.tile([P, NT], F32, tag="nbl")
    nc.vector.tensor_scalar(nbl[:], predf[:], 0.0, None, op0=OP.not_equal)
    nc.vector.tensor_tensor(nbq[:], nbq[:], vall[:], op=OP.mult)
    nc.vector.tensor_tensor(nb[:], nbq[:], nbl[:], op=OP.mult)

    # ---- fold p~ and nb into rows; prefix-scan for seg ids / p cumsum ----
    pnb = sb.tile([P, 2 * NT], F32, tag="pnb")
    nc.vector.tensor_copy(pnb[:, 0:NT], ptil[:])
    nc.vector.tensor_copy(pnb[:, NT:2 * NT], nb[:])
    ps32 = pt.tile([2 * NT, P], F32, tag="tp")
    nc.tensor.transpose(ps32[:], pnb[:], ident[:])
    pnbT = sb.tile([2 * NT, P], F32, tag="pnbT")
    nc.vector.tensor_copy(pnbT[:], ps32[:])

    prow = sbc.tile([1, T], F32, tag="prow")
    nrow = sbc.tile([1, T], F32, tag="nrow")
    nc.sync.dma_start(prow[:], pnbT[0:NT, :])
    nc.sync.dma_start(nrow[:], pnbT[NT:2 * NT, :])

    pcrow = sbc.tile([1, T], F32, tag="pcrow")
    nc.vector.tensor_tensor_scan(pcrow[:], prow[:], prow[:], initial=0.0,
                                 op0=OP.add, op1=OP.bypass)
    scrow = sbc.tile([1, T], F32, tag="scrow")
    nc.vector.tensor_tensor_scan(scrow[:], nrow[:], nrow[:], initial=0.0,
                                 op0=OP.add, op1=OP.bypass)

    seg16 = sbc.tile([P, NT], F32, tag="seg16")
    for c in range(NT):
        nc.scalar.dma_start(seg16[:, c:c + 1], scrow[0:1, P * c:P * (c + 1)])
    pccol16 = sbc.tile([P, NT], F32, tag="pccol16")
    for c in range(NT):
        nc.scalar.dma_start(pccol16[:, c:c + 1], pcrow[0:1, P * c:P * (c + 1)])

    # ---- scatter segment starts: a[seg_idx[t]] = t where nb[t] ----
    si = sb.tile([P, NT], F32, tag="si")
    nc.vector.tensor_scalar_add(si[:], seg16[:], -1.0)
    tr = sb.tile([P, NT], F32, tag="tr")
    nc.vector.tensor_scalar_add(tr[:], tiotaf[:], TRASH)
    idxf = sb.tile([P, NT], F32, tag="idxf")
    nc.vector.tensor_tensor(idxf[:], si[:], tr[:], op=OP.subtract)
    nc.vector.tensor_tensor(idxf[:], idxf[:], nb[:], op=OP.mult)
    nc.vector.tensor_tensor(idxf[:], idxf[:], tr[:], op=OP.add)
    idxi = sbc.tile([P, NT], I32, tag="idxi")
    nc.vector.tensor_copy(idxi[:], idxf[:])
    for c in range(NT):
        nc.gpsimd.indirect_dma_start(
            out=at[:, :],
            out_offset=IndirectOffsetOnAxis(ap=idxi[:, c:c + 1], axis=0),
            in_=tiota[:, c:c + 1],
            in_offset=None)

    # asb[p, c] = a[127*c + p]: each gather chunk holds 128 consecutive
    # segment starts, so all 127 adjacent diffs are intra-chunk (no seam fix)
    asb = sbc.tile([P, NT + 1], I32, tag="asb")
    for c in range(NT + 1):
        nc.scalar.dma_start(asb[:, c:c + 1], at[127 * c:127 * c + P, 0:1])

    # ---- global prefix scans along T (DVE only) ----
    for j in range(ND):
        g, h = divmod(j, 4)
        sl = zTp[g][:, T * h:T * (h + 1)]
        nc.vector.tensor_tensor_scan(sl, sl, sl, initial=0.0,
                                     op0=OP.add, op1=OP.bypass)

    # ---- per-group: transpose back, append p-col (group 0), write CE ----
    for i in range(NT):
        for pr in range(NPAIR):
            w = CW0 if pr == 0 else PW
            ce = sbot.tile([P, CW0], F32, tag="ce")
            if pr == 0:
                nc.gpsimd.memset(ce[:, PCOL + 1:CW0], 0.0)
            tp2 = pt.tile([P, 512], F32, tag="tp")
            for h in range(4):
                nc.tensor.transpose(tp2[:, P * h:P * (h + 1)],
                                    zTp[pr][:, T * h + P * i:T * h + P * (i + 1)],
                                    ident[:])
            if (i + pr) % 2 == 0:
                nc.vector.tensor_copy(ce[:, 0:PW], tp2[:])
            else:
                nc.scalar.copy(ce[:, 0:PW], tp2[:])
            if pr == 0:
                # pad cols (513..527) stay garbage — never read downstream
                nc.vector.tensor_copy(ce[:, PCOL:PCOL + 1],
                                      pccol16[:, i:i + 1])
            eng = nc.sync if pr == 0 else nc.scalar
            eng.dma_start(cets[pr][1 + P * i:1 + P * (i + 1), :], ce[:, 0:w])

    # ---- fallback + new_lengths scalars ----
    nsegv = scrow[0:1, T - 1:T]
    e00 = sb.tile([1, 1], F32, tag="e00")
    nc.scalar.activation(e00[:], l00[:], AF.Exp, bias=mcol[0:1, 0:1], scale=1.0)
    p0 = sb.tile([1, 1], F32, tag="p0")
    nc.vector.tensor_tensor(p0[:], e00[:], pcol[0:1, 0:1], op=OP.mult)
    pe0 = sb.tile([1, 1], F32, tag="pe0")
    nc.vector.tensor_scalar_add(pe0[:], p0[:], EPS)
    per0 = sb.tile([1, 1], F32, tag="per0")
    nc.vector.reciprocal(per0[:], pe0[:])
    fc = sb.tile([1, 1], F32, tag="fc")
    nc.vector.tensor_tensor(fc[:], p0[:], per0[:], op=OP.mult)
    e1 = sb.tile([1, 1], F32, tag="e1")
    nc.vector.tensor_scalar(e1[:], nsegv, 0.0, None, op0=OP.is_equal)
    e2 = sb.tile([1, 1], F32, tag="e2")
    nc.vector.tensor_scalar(e2[:], lent[0:1, 0:1], 1.0, None, op0=OP.is_ge)
    flag = sb.tile([1, 1], F32, tag="flag")
    nc.vector.tensor_tensor(flag[:], e1[:], e2[:], op=OP.mult)
    fcoef = sbc.tile([1, 1], F32, tag="fcoef")
    nc.vector.tensor_tensor(fcoef[:], fc[:], flag[:], op=OP.mult)
    fbrow = sbc.tile([1, D], F32, tag="fbrow")
    nc.vector.tensor_scalar_mul(fbrow[:], hrow0[:], fcoef[0:1, 0:1])

    nlf = sb.tile([1, 1], F32, tag="nlf")
    nc.vector.tensor_scalar_max(nlf[:], nsegv, 1.0)
    nli = sb.tile([1, 1], I32, tag="nli")
    nc.vector.tensor_copy(nli[:], nlf[:])
    nc.sync.dma_start(nlen[:, :], nli[:])

    # ---- gather G[s] = CE[a[s]] per pair, band-diff, normalize, store ----
    gts = {}
    for pr in range(NPAIR):
        w = CW0 if pr == 0 else PW
        for k in range(NT + 1):
            gt = sbg.tile([P, w], F32, tag=f"gt{pr}")
            nc.gpsimd.indirect_dma_start(
                out=gt[:], out_offset=None, in_=cets[pr][:, :],
                in_offset=IndirectOffsetOnAxis(ap=asb[:, k:k + 1], axis=0))
            gts[(pr, k)] = gt

    recs = {}
    for k in range(NT + 1):
        n = min(127, T - 127 * k)          # output rows this chunk
        obig = sbot.tile([P, D], F32, tag="obig")
        for pr in range(NPAIR):
            w = CW0 if pr == 0 else PW
            sp = ps.tile([P, CW0], F32, tag="sp")
            nc.tensor.matmul(sp[:, 0:512], lhsT=adiff[:],
                             rhs=gts[(pr, k)][:, 0:512], start=True, stop=True)
            if pr == 0:
                nc.tensor.matmul(sp[:, 512:CW0], lhsT=adiff[:],
                                 rhs=gts[(pr, k)][:, 512:CW0],
                                 start=True, stop=True)
                radd = sb.tile([P, 1], F32, tag="radd")
                nc.vector.tensor_scalar_add(radd[:], sp[:, PCOL:PCOL + 1], EPS)
                rec = sbot.tile([P, 1], F32, tag="rec")
                nc.vector.reciprocal(rec[:], radd[:])
                recs[k] = rec
            osl = obig[:, PW * pr:PW * (pr + 1)]
            if (k + pr) % 2 == 0:
                nc.scalar.mul(osl, sp[:, 0:PW], recs[k][:])
            else:
                nc.vector.tensor_scalar_mul(osl, sp[:, 0:PW], recs[k][:])
            if k == 0:
                nc.vector.tensor_tensor(obig[0:1, PW * pr:PW * (pr + 1)],
                                        obig[0:1, PW * pr:PW * (pr + 1)],
                                        fbrow[0:1, PW * pr:PW * (pr + 1)],
                                        op=OP.add)
        # write 128 rows (full-partition source spreads across all 16 SDMA
        # engines; 127-row sources collapse onto one). Row 127 of the chunk
        # is garbage but is overwritten by chunk k+1's row 0 — same HWDGE
        # queue, FIFO order guarantees the final value.
        n128 = min(P, T - 127 * k)
        nc.sync.dma_start(out[127 * k:127 * k + n128, :], obig[0:n128, :])


def build_nc():
    nc = bacc.Bacc("TRN2", target_bir_lowering=False, debug=False)
    lg = nc.dram_tensor("lg", [T, V], F32, kind="ExternalInput")
    hs = nc.dram_tensor("hs", [T, D], F32, kind="ExternalInput")
    lenb = nc.dram_tensor("lenb", [P, 1], F32, kind="ExternalInput")
    out = nc.dram_tensor("out", [T, D], F32, kind="ExternalOutput")
    nlen = nc.dram_tensor("nlen", [1, 1], I32, kind="ExternalOutput")
    with tile.TileContext(nc) as tc:
        with ExitStack() as ctx:
            _build_body(ctx, tc, nc, lg.ap(), hs.ap(), lenb.ap(), out.ap(),
                        nlen.ap())
    nc.compile()
    return nc


_NC = None


def _get_nc():
    global _NC
    if _NC is None:
        _NC = build_nc()
    return _NC


def make_in_maps(hidden_states, ctc_logits, lengths):
    in_maps = []
    for b in range(NCORES):
        in_maps.append({
            "lg": np.ascontiguousarray(ctc_logits[b], dtype=np.float32),
            "hs": np.ascontiguousarray(hidden_states[b], dtype=np.float32),
            "lenb": np.full((P, 1), float(lengths[b]), dtype=np.float32),
        })
    return in_maps


def kernel(hidden_states, ctc_logits, lengths, **run_kwargs):
    hidden_states = np.asarray(hidden_states)
    ctc_logits = np.asarray(ctc_logits)
    lengths = np.asarray(lengths)
    nc = _get_nc()
    in_maps = make_in_maps(hidden_states, ctc_logits, lengths)
    res = run_bass_kernel_spmd(nc, in_maps, core_ids=list(range(NCORES)),
                               **run_kwargs)
    compressed = np.stack([res.results[b]["out"] for b in range(NCORES)])
    new_lengths = np.array(
        [res.results[b]["nlen"].reshape(()) for b in range(NCORES)],
        dtype=np.int32)
    return compressed, new_lengths
